# revision 1
# baseline (speedup 1.0000x reference)
"""GuidedFilter Trainium2 kernel: batch-parallel over 8 NeuronCores.

Per core: img [1,512,512] f32, feat [16,512,512] f32 -> out [16,512,512] f32.
Each 2-D reflect box blur (radius 5) is two TensorE passes against a banded
unnormalized box matrix B (entries {0,1,2}, exact in bf16):
  pass A' (data-as-weights): T1 = X^T B^T   (contracts partition dim, flips orientation)
  pass C  (const weights):   out = B T1     (contracts partition dim again)
=> out = B X^T... = (B X B^T)^T = raw 2-D blur, transposed. The 1/121
normalization is folded into later elementwise ops. Orientations alternate so
no explicit transposes are needed anywhere.
"""
import sys

sys.path.insert(0, "/opt/trn_rl_repo")

import numpy as np
import ml_dtypes

RADIUS = 5
EPS = 1e-08
H = W = 512
D = 16
NCORES = 8
U = 1.0 / 121.0  # box normalization (11x11)

_BAND = [ [0, 1], [0, 1, 2], [1, 2, 3], [2, 3] ]  # band(j): i-blocks touched
_GJ_OFF = [0, 256, 640, 1024]                      # col offset of GJ[j] in packed G
_GJ_LEN = [256, 384, 384, 256]


def _box_matrix():
    B = np.zeros((512, 512), np.float32)
    for i in range(512):
        for d in range(-RADIUS, RADIUS + 1):
            j = i + d
            if j < 0:
                j = -j
            elif j > 511:
                j = 1022 - j
            B[i, j] += 1.0
    return B


def _g_packed():
    B = _box_matrix()
    cols = []
    for j in range(4):
        for i in _BAND[j]:
            cols.append(B[128 * i:128 * i + 128, 128 * j:128 * j + 128].T)
    return np.ascontiguousarray(np.concatenate(cols, axis=1)).astype(ml_dtypes.bfloat16)


def _build_bass():
    import concourse.bass as bass
    import concourse.bacc as bacc
    import concourse.tile as tile
    from concourse import mybir

    f32 = mybir.dt.float32
    bf16 = mybir.dt.bfloat16
    Alu = mybir.AluOpType
    Act = mybir.ActivationFunctionType

    nc = bacc.Bacc("TRN2", target_bir_lowering=False, debug=False,
                   num_devices=NCORES)

    feat_d = nc.dram_tensor("feat", [D, H, W], f32, kind="ExternalInput").ap()
    img_d = nc.dram_tensor("img", [1, H, W], f32, kind="ExternalInput").ap()
    g_d = nc.dram_tensor("gmat", [128, 1280], bf16, kind="ExternalInput").ap()
    out_d = nc.dram_tensor("out", [D, H, W], f32, kind="ExternalOutput").ap()

    def ld(dst, src2d):
        # HBM [512,512] f32 -> SBUF [128, 4*512] (j-chunk major), cast to bf16
        nc.gpsimd.dma_start(
            out=dst.rearrange("p (j w) -> p j w", j=4),
            in_=src2d.rearrange("(j p) w -> p j w", p=128))

    with tile.TileContext(nc) as tc:
        with (
            tc.tile_pool(name="consts", bufs=1) as consts,
            tc.tile_pool(name="shared", bufs=1) as shared,
            tc.tile_pool(name="chan", bufs=2) as chan,
            tc.tile_pool(name="psum", bufs=1, space="PSUM") as psum,
        ):
            G = consts.tile([128, 1280], bf16)
            nc.gpsimd.dma_start(out=G[:], in_=g_d)
            I = consts.tile([128, 2048], bf16)
            ld(I, img_d[0])

            def ap_blur(X):
                """pass A': psum tiles T1 (list of 4 [128,512] f32) = X^T B^T."""
                ps = [psum.tile([128, 512], f32, name=f"psA{wb}", tag=f"psA{wb}") for wb in range(4)]
                for j in range(4):
                    i0 = _BAND[j][0]
                    ilen = len(_BAND[j])
                    rhs = G[:, _GJ_OFF[j]:_GJ_OFF[j] + _GJ_LEN[j]]
                    for wb in range(4):
                        lhsT = X[:, 512 * j + 128 * wb: 512 * j + 128 * (wb + 1)]
                        nc.tensor.matmul(
                            ps[wb][:, 128 * i0: 128 * (i0 + ilen)],
                            lhsT, rhs, start=(j == 0), stop=(j == 3),
                            skip_group_check=True)
                return ps

            def handoff(ps, engines="AAAA"):
                """psum A' tiles -> one [128,2048] bf16 sbuf tile."""
                t = chan.tile([128, 2048], bf16, tag="t1")
                for wb in range(4):
                    dst = t[:, 512 * wb: 512 * (wb + 1)]
                    if engines[wb] == "A":
                        nc.scalar.copy(dst, ps[wb][:])
                    else:
                        nc.vector.tensor_copy(dst, ps[wb][:])
                return t

            def c_blur(T1, tag):
                """pass C: psum tiles out[i] [128,512] f32 = (B @ T1) chunks."""
                ps = [psum.tile([128, 512], f32, name=f"psC{i}", tag=f"psC{i}") for i in range(4)]
                for i in range(4):
                    band = _BAND[i]  # symmetric: band(i) as j-range equals _BAND[i]
                    for pos, j in enumerate(band):
                        off = _GJ_OFF[j] + 128 * (i - _BAND[j][0])
                        lhsT = G[:, off:off + 128]
                        rhs = T1[:, 512 * j: 512 * (j + 1)]
                        nc.tensor.matmul(
                            ps[i][:], lhsT, rhs,
                            start=(pos == 0), stop=(pos == len(band) - 1))
                return ps

            def blur2(X):
                return c_blur(handoff(ap_blur(X)), tag="c")

            # ---- shared (img) stage; all blur outputs are [W,H]-oriented ----
            I2 = shared.tile([128, 2048], bf16)
            nc.vector.tensor_mul(I2[:], I[:], I[:])
            psI = blur2(I)   # mI_raw^T
            mIs = shared.tile([128, 2048], bf16)   # true-scale mean_I^T
            for wb in range(4):
                nc.scalar.activation(mIs[:, 512 * wb:512 * (wb + 1)], psI[wb][:],
                                     Act.Copy, 0.0, U)
            psI2 = blur2(I2)  # corrI_raw^T
            m2 = shared.tile([128, 2048], f32)
            nc.vector.tensor_mul(m2[:], mIs[:], mIs[:])
            varps = shared.tile([128, 2048], f32)
            for wb in range(4):
                sl = slice(512 * wb, 512 * (wb + 1))
                # var = U*corrI_raw - mI^2   (merged psum evac)
                nc.vector.scalar_tensor_tensor(
                    varps[:, sl], psI2[wb][:], U, m2[:, sl],
                    op0=Alu.mult, op1=Alu.subtract)
            nc.vector.tensor_scalar_add(varps[:], varps[:], EPS)
            R = shared.tile([128, 2048], f32)
            nc.vector.reciprocal_approx_fast(R[:], varps[:])
            RS = shared.tile([128, 2048], bf16)
            nc.vector.tensor_scalar_mul(RS[:], R[:], U)     # U * R
            mIR = shared.tile([128, 2048], bf16)
            nc.vector.tensor_mul(mIR[:], mIs[:], R[:])      # mI * R

            # ---- per feature channel ----
            for d in range(D):
                Xd = chan.tile([128, 2048], bf16, tag="xd")
                ld(Xd, feat_d[d])
                Pd = chan.tile([128, 2048], bf16, tag="pd")
                nc.gpsimd.tensor_mul(Pd[:], Xd[:], I[:])

                ps_mp = blur2(Xd)       # mp_raw^T
                mp = chan.tile([128, 2048], bf16, tag="mp")
                for wb in range(4):     # true-scale mp (ACT, scale=U)
                    nc.scalar.activation(mp[:, 512 * wb:512 * (wb + 1)],
                                         ps_mp[wb][:], Act.Copy, 0.0, U)
                ps_cip = blur2(Pd)      # corrIp_raw^T
                t2 = chan.tile([128, 2048], bf16, tag="t2")
                for wb in range(4):     # t2 = corrIp * R  (merged evac)
                    sl = slice(512 * wb, 512 * (wb + 1))
                    nc.vector.tensor_mul(t2[:, sl], ps_cip[wb][:], RS[:, sl])
                t1 = chan.tile([128, 2048], bf16, tag="t1m")
                nc.vector.tensor_mul(t1[:], mp[:], mIR[:])
                a = chan.tile([128, 2048], bf16, tag="a")
                nc.vector.tensor_sub(a[:], t2[:], t1[:])
                u2 = chan.tile([128, 2048], bf16, tag="u2")
                nc.gpsimd.tensor_mul(u2[:], a[:], mIs[:])
                b = chan.tile([128, 2048], bf16, tag="b")
                nc.vector.tensor_sub(b[:], mp[:], u2[:])

                ps_ma = blur2(a)        # ma_raw, [H,W] again
                v = chan.tile([128, 2048], f32, tag="v")
                for wb in range(4):     # v = (U^2 * ma_raw) * I   (a true-scale => /121 once... )
                    sl = slice(512 * wb, 512 * (wb + 1))
                    nc.vector.scalar_tensor_tensor(
                        v[:, sl], ps_ma[wb][:], U, I[:, sl],
                        op0=Alu.mult, op1=Alu.mult)
                ps_mb = blur2(b)        # mb_raw
                o = chan.tile([128, 2048], bf16, tag="o")
                for wb in range(4):     # o = U*mb_raw + v
                    sl = slice(512 * wb, 512 * (wb + 1))
                    nc.vector.scalar_tensor_tensor(
                        o[:, sl], ps_mb[wb][:], U, v[:, sl],
                        op0=Alu.mult, op1=Alu.add)
                nc.gpsimd.dma_start(
                    out=out_d[d].rearrange("(j p) w -> p j w", p=128),
                    in_=o.rearrange("p (j w) -> p j w", j=4))

    nc.compile()
    return nc


_NC_CACHE = None


def kernel(feat: np.ndarray, img: np.ndarray) -> np.ndarray:
    global _NC_CACHE
    from concourse.bass_utils import run_bass_kernel_spmd

    if _NC_CACHE is None:
        _NC_CACHE = _build_bass()
    nc = _NC_CACHE
    g = _g_packed()
    feat = np.asarray(feat, np.float32)
    img = np.asarray(img, np.float32)
    in_maps = [
        {"feat": feat[c], "img": img[c], "gmat": g} for c in range(NCORES)
    ]
    res = run_bass_kernel_spmd(nc, in_maps, list(range(NCORES)))
    return np.stack([res.results[c]["out"] for c in range(NCORES)], axis=0)



# revision 7
# speedup vs baseline: 1.5910x; 1.5910x over previous
"""GuidedFilter Trainium2 kernel v2: batch-parallel over 8 NeuronCores.

Per core: img [512,512] bf16, feat [16,512,512] bf16 -> out [16,512,512] bf16
(host casts f32<->bf16). Each 2-D box blur (radius 5, reflect) is two PE
passes against per-128-chunk diagonal blocks of the box matrix B plus tiny
5-wide boundary-correction matmuls, cutting PE streaming 2.4x vs a banded
block decomposition:
  pass1: T1[w,i] = sum_r X[r,w] B[i,r]   (lhsT = X chunks, rhs = B^T blocks)
  pass2: out[i,w'] = sum_w T1[w,i] B[w',w]
Orientation is preserved (out has the input layout). Evacs are fused with the
per-channel elementwise math and spread across Act/DVE/Pool; data DMAs issue
from the idle SP (sync) HWDGE path. PSUM rotates two [128,2048] f32 tiles
(4 banks each); emission software-pipelines phase1(d) with phase2(d-1).
"""
import sys

sys.path.insert(0, "/opt/trn_rl_repo")

import numpy as np
import ml_dtypes

RADIUS = 5
H = W = 512
D = 16
NCORES = 8
U = 1.0 / 121.0
VAR_FLOOR = 1e-6

_BT_OFF = [0, 128, 128, 256]  # rhs col offset of B^T diag block per chunk
_TRI_NEXT = 384               # [5,5] correction blocks
_TRI_PREV = 392


def _box_matrix():
    B = np.zeros((512, 512), np.float32)
    for i in range(512):
        for d in range(-RADIUS, RADIUS + 1):
            j = i + d
            if j < 0:
                j = -j
            elif j > 511:
                j = 1022 - j
            B[i, j] += 1.0
    return B


def _g_packed():
    B = _box_matrix()
    G = np.zeros((128, 512), np.float32)
    G[:, 0:128] = B[0:128, 0:128].T
    G[:, 128:256] = B[128:256, 128:256].T
    G[:, 256:384] = B[384:512, 384:512].T
    # tri_next[s,o] = B[123+o, 128+s]; tri_prev[s,o] = B[128+o, 123+s].
    # tri_prev sits at partitions 123:128 so its matmuls can use a legal
    # base-64 partition slice (PE requires operand base in {0,32,64}).
    G[0:5, 384:389] = B[123:128, 128:133].T
    G[123:128, 392:397] = B[128:133, 123:128].T
    return np.ascontiguousarray(G).astype(ml_dtypes.bfloat16)


def _build_bass():
    import concourse.bass as bass
    import concourse.bacc as bacc
    import concourse.tile as tile
    from concourse import mybir

    f32 = mybir.dt.float32
    bf16 = mybir.dt.bfloat16
    Alu = mybir.AluOpType
    Act = mybir.ActivationFunctionType

    nc = bacc.Bacc("TRN2", target_bir_lowering=False, debug=False,
                   num_devices=NCORES)

    feat_d = nc.dram_tensor("feat", [D, H, W], bf16, kind="ExternalInput").ap()
    img_d = nc.dram_tensor("img", [H, W], bf16, kind="ExternalInput").ap()
    g_d = nc.dram_tensor("gmat", [128, 512], bf16, kind="ExternalInput").ap()
    out_d = nc.dram_tensor("out", [D, H, W], bf16, kind="ExternalOutput").ap()

    def ld(dst, src2d):
        nc.sync.dma_start(
            out=dst.rearrange("p (j w) -> p j w", j=4),
            in_=src2d.rearrange("(j p) w -> p j w", p=128))

    def st(dst2d, src):
        nc.sync.dma_start(
            out=dst2d.rearrange("(j p) w -> p j w", p=128),
            in_=src.rearrange("p (j w) -> p j w", j=4))

    with tile.TileContext(nc) as tc:
        with (
            tc.tile_pool(name="consts", bufs=1) as consts,
            tc.tile_pool(name="shared", bufs=1) as shared,
            tc.tile_pool(name="xin", bufs=3) as xin,
            tc.tile_pool(name="chan", bufs=2) as chan,
            tc.tile_pool(name="t1p", bufs=4) as t1p,
            tc.tile_pool(name="psum", bufs=2, space="PSUM") as psum,
        ):
            G = consts.tile([128, 512], bf16)
            nc.sync.dma_start(out=G[:], in_=g_d)
            I = consts.tile([128, 2048], bf16)
            ld(I, img_d)

            def pass1(Xt, P1):
                """P1[w-chunk c][:, i] = blur-rows of Xt (both [128,2048])."""
                for c in range(4):
                    base = 512 * c
                    for j in range(4):
                        nc.tensor.matmul(
                            P1[:, base + 128 * j: base + 128 * (j + 1)],
                            Xt[:, 512 * j + 128 * c: 512 * j + 128 * c + 128],
                            G[:, _BT_OFF[j]: _BT_OFF[j] + 128],
                            start=(j == 0), stop=False, skip_group_check=True)
                    for j in range(3):
                        nc.tensor.matmul(
                            P1[:, base + 128 * j + 123: base + 128 * (j + 1)],
                            Xt[0:5, 512 * (j + 1) + 128 * c: 512 * (j + 1) + 128 * c + 128],
                            G[0:5, _TRI_NEXT:_TRI_NEXT + 5],
                            start=False, stop=False, skip_group_check=True)
                        nc.tensor.matmul(
                            P1[:, base + 128 * (j + 1): base + 128 * (j + 1) + 5],
                            Xt[64:128, 512 * j + 128 * c: 512 * j + 128 * c + 128],
                            G[64:128, _TRI_PREV:_TRI_PREV + 5],
                            start=False, stop=(j == 2), skip_group_check=True)

            def pass2(T1, P2):
                """P2[i-chunk k][:, w'] = blur-cols of T1."""
                for k in range(4):
                    base = 512 * k
                    for c in range(4):
                        nc.tensor.matmul(
                            P2[:, base + 128 * c: base + 128 * (c + 1)],
                            T1[:, 512 * c + 128 * k: 512 * c + 128 * k + 128],
                            G[:, _BT_OFF[c]: _BT_OFF[c] + 128],
                            start=(c == 0), stop=False, skip_group_check=True)
                    for c in range(3):
                        nc.tensor.matmul(
                            P2[:, base + 128 * c + 123: base + 128 * (c + 1)],
                            T1[0:5, 512 * (c + 1) + 128 * k: 512 * (c + 1) + 128 * k + 128],
                            G[0:5, _TRI_NEXT:_TRI_NEXT + 5],
                            start=False, stop=False, skip_group_check=True)
                        nc.tensor.matmul(
                            P2[:, base + 128 * (c + 1): base + 128 * (c + 1) + 5],
                            T1[64:128, 512 * c + 128 * k: 512 * c + 128 * k + 128],
                            G[64:128, _TRI_PREV:_TRI_PREV + 5],
                            start=False, stop=(c == 2), skip_group_check=True)

            # ---------------- img stage ----------------
            xtiles = {}
            for dd in range(min(2, D)):
                Xt = xin.tile([128, 2048], bf16, tag="x", name=f"x{dd}")
                ld(Xt, feat_d[dd])
                xtiles[dd] = Xt

            I2 = shared.tile([128, 2048], bf16)
            nc.vector.tensor_mul(I2[:], I[:], I[:])

            P1i = psum.tile([128, 2048], f32, tag="ps", name="p1i")
            pass1(I, P1i)
            T1i = t1p.tile([128, 2048], bf16, tag="t1", name="t1i")
            nc.scalar.copy(T1i[:], P1i[:])
            P2i = psum.tile([128, 2048], f32, tag="ps", name="p2i")
            pass2(T1i, P2i)
            mIs = shared.tile([128, 2048], bf16)
            nc.scalar.activation(mIs[:], P2i[:], Act.Copy, 0.0, U)

            P1j = psum.tile([128, 2048], f32, tag="ps", name="p1j")
            pass1(I2, P1j)
            T1j = t1p.tile([128, 2048], bf16, tag="t1", name="t1j")
            nc.scalar.copy(T1j[:], P1j[:])
            P2j = psum.tile([128, 2048], f32, tag="ps", name="p2j")
            pass2(T1j, P2j)

            m2 = shared.tile([128, 2048], f32)
            nc.vector.tensor_mul(m2[:], mIs[:], mIs[:])
            varp = shared.tile([128, 2048], f32)
            nc.vector.scalar_tensor_tensor(
                varp[:], P2j[:], U, m2[:], op0=Alu.mult, op1=Alu.subtract)
            nc.vector.tensor_scalar_max(varp[:], varp[:], VAR_FLOOR)
            R = shared.tile([128, 2048], f32)
            nc.vector.reciprocal_approx_fast(R[:], varp[:])
            RS = shared.tile([128, 2048], bf16)
            nc.vector.tensor_scalar_mul(RS[:], R[:], U)
            mIR = shared.tile([128, 2048], bf16)
            nc.vector.tensor_mul(mIR[:], mIs[:], R[:])

            pd_t = {}
            Pd0 = chan.tile([128, 2048], bf16, tag="pd", name="pd0")
            nc.gpsimd.tensor_mul(Pd0[:], xtiles[0][:], I[:])
            pd_t[0] = Pd0

            mp_t, t2_t, v_t = {}, {}, {}

            def phase1(d):
                if d + 1 < D:
                    Xn = xin.tile([128, 2048], bf16, tag="x", name=f"x{d+1}")
                    ld(Xn, feat_d[d + 1])
                    xtiles[d + 1] = Xn
                X = xtiles[d]
                Pd = pd_t[d]
                P1x = psum.tile([128, 2048], f32, tag="ps", name=f"p1x{d}")
                pass1(X, P1x)
                P1p = psum.tile([128, 2048], f32, tag="ps", name=f"p1p{d}")
                pass1(Pd, P1p)
                T1x = t1p.tile([128, 2048], bf16, tag="t1", name=f"t1x{d}")
                nc.scalar.copy(T1x[:], P1x[:])              # H_X  (Act)
                T1q = t1p.tile([128, 2048], bf16, tag="t1", name=f"t1q{d}")
                nc.scalar.copy(T1q[:], P1p[:])              # H_P  (Act)
                P2x = psum.tile([128, 2048], f32, tag="ps", name=f"p2x{d}")
                pass2(T1x, P2x)
                P2p = psum.tile([128, 2048], f32, tag="ps", name=f"p2p{d}")
                pass2(T1q, P2p)
                mp = chan.tile([128, 2048], bf16, tag="mp", name=f"mp{d}")
                nc.scalar.activation(mp[:], P2x[:], Act.Copy, 0.0, U)  # E_mp (Act)
                mp_t[d] = mp
                return P2p

            ab_t = {}

            def phase2_chain(d):
                mp, t2 = mp_t[d], t2_t[d]
                t1m = chan.tile([128, 2048], bf16, tag="t1m", name=f"t1m{d}")
                nc.vector.tensor_mul(t1m[:], mp[:], mIR[:])
                a = chan.tile([128, 2048], bf16, tag="a", name=f"a{d}")
                nc.vector.tensor_sub(a[:], t2[:], t1m[:])
                u2 = chan.tile([128, 2048], bf16, tag="u2", name=f"u2{d}")
                nc.gpsimd.tensor_mul(u2[:], a[:], mIs[:])   # Pool
                b = chan.tile([128, 2048], bf16, tag="b", name=f"b{d}")
                nc.vector.tensor_sub(b[:], mp[:], u2[:])
                ab_t[d] = (a, b)

            def phase1_tail(d, P2p):
                t2 = chan.tile([128, 2048], bf16, tag="t2", name=f"t2{d}")
                nc.vector.tensor_mul(t2[:], P2p[:], RS[:])  # E_t2 (DVE)
                t2_t[d] = t2

            def phase2_blur(d):
                a, b = ab_t[d]
                P1a = psum.tile([128, 2048], f32, tag="ps", name=f"p1a{d}")
                pass1(a, P1a)
                P1b = psum.tile([128, 2048], f32, tag="ps", name=f"p1b{d}")
                pass1(b, P1b)
                T1a = t1p.tile([128, 2048], bf16, tag="t1", name=f"t1a{d}")
                nc.scalar.copy(T1a[:], P1a[:])              # H_a  (Act)
                T1b = t1p.tile([128, 2048], bf16, tag="t1", name=f"t1b{d}")
                nc.scalar.copy(T1b[:], P1b[:])              # H_b  (Act)
                P2a = psum.tile([128, 2048], f32, tag="ps", name=f"p2a{d}")
                pass2(T1a, P2a)
                P2b = psum.tile([128, 2048], f32, tag="ps", name=f"p2b{d}")
                pass2(T1b, P2b)
                v = chan.tile([128, 2048], bf16, tag="v", name=f"v{d}")
                nc.vector.scalar_tensor_tensor(
                    v[:], P2a[:], U, I[:], op0=Alu.mult, op1=Alu.mult)  # E_v (DVE)
                o = chan.tile([128, 2048], bf16, tag="o", name=f"o{d}")
                nc.vector.scalar_tensor_tensor(
                    o[:], P2b[:], U, v[:], op0=Alu.mult, op1=Alu.add)   # E_o (DVE)
                st(out_d[d], o)

            def prefetch_pd(d):
                if d < D:
                    Pd = chan.tile([128, 2048], bf16, tag="pd", name=f"pd{d}")
                    nc.gpsimd.tensor_mul(Pd[:], xtiles[d][:], I[:])  # Pool
                    pd_t[d] = Pd

            for d in range(D):
                P2p = phase1(d)
                if d > 0:
                    phase2_chain(d - 1)
                phase1_tail(d, P2p)
                if d > 0:
                    phase2_blur(d - 1)
                prefetch_pd(d + 1)
            phase2_chain(D - 1)
            phase2_blur(D - 1)

    nc.compile()
    return nc


_NC_CACHE = None


def kernel(feat: np.ndarray, img: np.ndarray) -> np.ndarray:
    global _NC_CACHE
    from concourse.bass_utils import run_bass_kernel_spmd

    if _NC_CACHE is None:
        _NC_CACHE = _build_bass()
    nc = _NC_CACHE
    g = _g_packed()
    bf = ml_dtypes.bfloat16
    featb = np.ascontiguousarray(np.asarray(feat, np.float32)).astype(bf)
    imgb = np.ascontiguousarray(np.asarray(img, np.float32)).astype(bf)
    in_maps = [
        {"feat": featb[c], "img": imgb[c, 0], "gmat": g} for c in range(NCORES)
    ]
    res = run_bass_kernel_spmd(nc, in_maps, list(range(NCORES)))
    return np.stack(
        [res.results[c]["out"].astype(np.float32) for c in range(NCORES)], axis=0)
